# revision 22
# baseline (speedup 1.0000x reference)
"""DeepseekV3 top-k router kernel for Trainium2 (Bass/Tile), 8-core SPMD.

Reference computation (per token, 256 experts):
  s    = sigmoid(logits)
  s4c  = s + correction_bias
  group scores = sum of top-2 of s4c within each of 8 groups of 32
  top-4 groups -> mask -> masked s4c
  top-8 of masked s4c -> (indices, values)
  weights = s at those indices, normalized to sum 2.5

Sharding: data-parallel on the token dim across 8 cores (16384 tokens each).
Layout: 8 tokens per SBUF partition row, 256 expert scores per token along
the free dim; supertile = [128, 2048].

Final design (411us baseline -> 353us):
  - index-encoded Max8: a fused custom DVE op (RTR_IDXENC) rewrites the
    low mantissa byte of each masked score with its expert id
    (enc = m ^ ((m ^ e) & 0xff), one pass), so Max8 alone returns
    value AND index; FindIndex8 + MATCH_VALUE_LOADs are gone.  The
    <= 2^-15 relative perturbation flips a near-tied expert on ~0.14%
    of tokens (idx rel err 1.4e-2, within the 2e-2 gate; w err 2.3e-3).
  - fused group score: custom op RTR_EQADD computes
    (s4c == max1) ? -FLT_MAX : s4c + max1 in one pass, so one segmented
    reduce yields max1+max2 directly (no separate m2 + add).
  - weights = v8 - bias16[idx]: the Max8 values ARE s4c at the selected
    experts, so scattering the constant bias row through the inverse map
    (local_scatter x2) recovers s exactly -- no second sigmoid, no
    data-dependent scatter payload.
  - s4c = sigmoid + bias via TensorE: ACT sigmoid -> SBUF, identity
    matmul + rank-1 ones x biasrow accumulate into PSUM (keeps the DVE
    out of the bias add; numerically bit-identical to the DVE add).
  - GPSIMD runs ONLY local_scatter: mixing Q7 tensor ops with scatters
    swaps the Q7 IRAM library (~2-6us per reload).  Small DVE ops stall
    behind in-flight scatters, so all tensor work stays on DVE where
    wide ops are immune.
  - 5-deep software pipeline: load+sigmoid+s4c/group block 2 ahead,
    ranks/mask 1 ahead, enc+Max8 current, scatters 1 behind,
    normalize+store 2 behind.
  """
import numpy as np

import concourse.bass as bass
import concourse.bacc as bacc
import concourse.mybir as mybir
from concourse.tile import TileContext
from concourse.bass_utils import run_bass_kernel_spmd

F32 = mybir.dt.float32
U16 = mybir.dt.uint16
F16 = mybir.dt.float16
I16 = mybir.dt.int16
U8 = mybir.dt.uint8

T_FULL = 131072
E = 256
N_CORES = 8
T_CORE = T_FULL // N_CORES      # 16384
P = 128                         # partition rows
B = 8                           # tokens per row (tiles per supertile)
N_TILES = T_CORE // P           # 128
N_SUPER = N_TILES // B          # 16
G = 8                           # expert groups per token
EG = E // G                     # experts per group
W = B * E                       # 2048 cols per supertile

SIM_SAFE_SUB = False            # CoreSim rejects duplicate scatter idxs

LAST_EXEC_NS = None
LAST_RESULTS = None

_EQNEG = None
_ENC = None
IMM_FF = float(np.frombuffer(np.uint32(255).tobytes(), np.float32)[0])


def _register(op_ctor):
    from concourse.dve_ops import (DveOp, OPS, get_dve_sub_opcode, has_src1)
    from concourse.dve_spec import lower
    from concourse.dve_uop import DveOpSpec
    import concourse.dve_ops as dve_ops_mod

    name, spec = op_ctor()
    op = DveOp(name, spec, subdim=False, uops_sha={})
    OPS.append(op)
    dve_ops_mod.CUSTOM_DVE_SPECS[op.name] = op.spec
    dve_ops_mod._SUB_OPCODE_FOR_NAME[op.name] = (
        dve_ops_mod._CUSTOM_DVE_ROW_BASE + len(OPS) - 1)
    assert dve_ops_mod._SUB_OPCODE_FOR_NAME[op.name] < 0x20
    for ver in ("v3", "v4"):
        tmp = DveOpSpec(name=op.name, opcode=get_dve_sub_opcode(op.name),
                        uops=lower(spec, ver=ver), rd1_en=has_src1(spec))
        op.uops_sha[ver] = tmp.sha(ver)
    return op


def _get_eqadd():
    """out = (in0 == in1) ? -FLT_MAX : in0 + in1 — argmax exclusion fused
    with the +max1 shift, so segmax(out) = max1 + max2 (the group score)."""
    global _EQNEG
    if _EQNEG is None:
        def ctor():
            from concourse.dve_spec import Spec, Src0, Src1, MaxNeg, select, eq

            def ref(in0, in1, s0, s1, imm2):
                b = np.broadcast_to(np.asarray(in1, np.float32),
                                    np.asarray(in0).shape)
                return np.where(
                    in0 == b, np.float32(-3.4028234663852886e38),
                    (np.asarray(in0, np.float32) + b)).astype(np.float32)

            return "RTR_EQADD", Spec(
                body=select(eq(Src0, Src1), MaxNeg, Src0 + Src1),
                reference=ref,
            )
        _EQNEG = _register(ctor)
    return _EQNEG


def _get_enc():
    """out = in0 with its low mantissa byte replaced by in1's low byte:
    enc = x ^ ((x ^ e) & 0xff) — Max8 on enc returns value AND index."""
    global _ENC
    if _ENC is None:
        def ctor():
            from concourse.dve_spec import Spec, Src0, Src1, C2, Bin, AluOp

            def ref(in0, in1, s0, s1, imm2):
                a = np.ascontiguousarray(np.asarray(in0, np.float32))
                b = np.ascontiguousarray(
                    np.broadcast_to(np.asarray(in1, np.float32), a.shape))
                ab = a.view(np.int32)
                bb = b.view(np.int32)
                m = np.array(imm2, np.float32).view(np.int32)[()]
                return (ab ^ ((ab ^ bb) & m)).view(np.float32)

            body = Bin(AluOp.BITWISE_XOR, Src0,
                       Bin(AluOp.BITWISE_AND,
                           Bin(AluOp.BITWISE_XOR, Src0, Src1), C2))
            return "RTR_IDXENC", Spec(body=body, reference=ref)
        _ENC = _register(ctor)
    return _ENC


def _build(nc: bass.Bass):
    x_d = nc.dram_tensor("logits", [T_CORE, E], F32, kind="ExternalInput")
    b_d = nc.dram_tensor("bias", [1, W], F32, kind="ExternalInput")
    id_d = nc.dram_tensor("ident", [P, P], F32, kind="ExternalInput")
    on_d = nc.dram_tensor("ones1", [1, P], F32, kind="ExternalInput")
    b16_d = nc.dram_tensor("bias16", [1, W], F16, kind="ExternalInput")
    erow_d = nc.dram_tensor("erow", [1, W], F32, kind="ExternalInput")
    offs_d = nc.dram_tensor("offs", [1, B * 8], U16, kind="ExternalInput")
    slot_d = nc.dram_tensor("slotdat", [1, B * 8], I16, kind="ExternalInput")
    idx_d = nc.dram_tensor("idx_out", [T_CORE, 8], U16, kind="ExternalOutput")
    w_d = nc.dram_tensor("w_out", [T_CORE, 8], F32, kind="ExternalOutput")

    AX = mybir.AxisListType.X
    OP = mybir.AluOpType
    ACTF = mybir.ActivationFunctionType
    eqadd = _get_eqadd()
    encop = _get_enc()

    with TileContext(nc) as tc:
        with tc.tile_pool(name="const", bufs=1) as cpool, \
             tc.tile_pool(name="io", bufs=2) as iopool, \
             tc.tile_pool(name="wideA", bufs=2) as wpool, \
             tc.tile_pool(name="s4cp", bufs=2, space="PSUM") as s4pool, \
             tc.tile_pool(name="slot", bufs=3) as slpool, \
             tc.tile_pool(name="out", bufs=4) as outpool:

            brow = cpool.tile([1, W], F32)
            nc.gpsimd.dma_start(out=brow[:], in_=b_d[:, :])
            identb = cpool.tile([P, P], F32)
            nc.gpsimd.dma_start(out=identb[:], in_=id_d[:, :])
            onesb = cpool.tile([1, P], F32)
            nc.gpsimd.dma_start(out=onesb[:], in_=on_d[:, :])
            biasb16 = cpool.tile([P, W], F16)
            nc.gpsimd.dma_start(out=biasb16[:],
                                in_=b16_d[:, :].to_broadcast((P, W)))
            erowb = cpool.tile([P, W], F32)
            nc.gpsimd.dma_start(out=erowb[:],
                                in_=erow_d[:, :].to_broadcast((P, W)))
            offsb = cpool.tile([P, B * 8], U16)
            nc.gpsimd.dma_start(out=offsb[:],
                                in_=offs_d[:, :].to_broadcast((P, B * 8)))
            slotb = cpool.tile([P, B * 8], I16)
            nc.gpsimd.dma_start(out=slotb[:],
                                in_=slot_d[:, :].to_broadcast((P, B * 8)))

            st = {}

            def stage_load(sp):
                srow = sp * B * P
                Ls = iopool.tile([P, W], F32, tag="L")
                nc.sync.dma_start(
                    out=Ls[:],
                    in_=x_d[srow:srow + B * P, :].rearrange(
                        "(p x) e -> p (x e)", p=P))
                s32 = wpool.tile([P, W], F32, tag="s32")
                nc.scalar.activation(s32[:], Ls[:], ACTF.Sigmoid)
                s4p = s4pool.tile([P, W], F32, tag="s4p")
                for n0 in range(0, W, 512):
                    nc.tensor.matmul(s4p[:, n0:n0 + 512], identb[:],
                                     s32[:, n0:n0 + 512],
                                     start=True, stop=False)
                for n0 in range(0, W, 512):
                    nc.tensor.matmul(s4p[:, n0:n0 + 512], onesb[:],
                                     brow[:, n0:n0 + 512],
                                     start=False, stop=True,
                                     skip_group_check=True)
                st[sp] = dict(s32=s32, s4p=s4p)

            def stage_f1a(sp):
                """DVE block: s4c = s + bias; per-group max and 2nd max."""
                d = st[sp]
                s4p = d["s4p"]
                s4c = wpool.tile([P, W], F32, tag="s4c")
                nc.scalar.activation(s4c[:], s4p[:], ACTF.Copy)
                m1 = slpool.tile([P, B * G], F32, tag="m1")
                nc.vector.tensor_reduce(
                    m1[:].rearrange("p (b g) -> p b g", b=B),
                    s4c[:].rearrange("p (b g e) -> p b g e", b=B, g=G),
                    axis=AX, op=OP.max)
                t2 = wpool.tile([P, W], F32, tag="t2")
                nc.vector._custom_dve(
                    eqadd,
                    out=t2[:].rearrange("p (q e) -> p q e", q=B * G),
                    in0=s4c[:].rearrange("p (q e) -> p q e", q=B * G),
                    in1=m1[:].rearrange("p q -> p q", q=B * G)
                        .unsqueeze(2).broadcast_to([P, B * G, EG]))
                gss = slpool.tile([P, B * G], F32, tag="gss")
                nc.vector.tensor_reduce(
                    gss[:].rearrange("p (b g) -> p b g", b=B),
                    t2[:].rearrange("p (b g e) -> p b g e", b=B, g=G),
                    axis=AX, op=OP.max)
                d.update(s4c=s4c, gss=gss)

            def stage_ranks(sp):
                """DVE: 8x8 rank compare + rank sum."""
                d = st[sp]
                gs3 = d["gss"][:].rearrange("p (b g) -> p b g", b=B)
                gts = slpool.tile([P, B * G * G], F32, tag="gts")
                nc.vector.tensor_tensor(
                    gts[:].rearrange("p (b i j) -> p b i j", b=B, i=G),
                    gs3.unsqueeze(2).broadcast_to([P, B, G, G]),
                    gs3.unsqueeze(3).broadcast_to([P, B, G, G]),
                    op=OP.is_gt)
                ranks = slpool.tile([P, B * G], F32, tag="ranks")
                nc.vector.tensor_reduce(
                    ranks[:],
                    gts[:].rearrange("p (b i j) -> p b i j", b=B, i=G),
                    axis=AX, op=OP.add)
                d["ranks"] = ranks

            def stage_mask(sp):
                """DVE: masked = (rank < 4) * s4c in one stt pass."""
                d = st[sp]
                masked = wpool.tile([P, W], F32, tag="masked")
                nc.vector.scalar_tensor_tensor(
                    masked[:].rearrange("p (q e) -> p q e", q=B * G),
                    d["ranks"][:].rearrange("p q -> p q", q=B * G)
                        .unsqueeze(2).broadcast_to([P, B * G, EG]),
                    4.0,
                    d["s4c"][:].rearrange("p (q e) -> p q e", q=B * G),
                    op0=OP.is_lt, op1=OP.mult)
                d["masked"] = masked

            def stage_f2(sp):
                """DVE: index-encode + per-token Max8; extract expert ids."""
                d = st[sp]
                enc = wpool.tile([P, W], F32, tag="enc")
                nc.vector._custom_dve(
                    encop, out=enc[:], in0=d["masked"][:], in1=erowb[:],
                    imm2=IMM_FF)
                v8e = outpool.tile([P, B * 8], F32, tag="v8e")
                for b in range(B):
                    nc.vector.max(out=v8e[:, b * 8:(b + 1) * 8],
                                  in_=enc[:, b * E:(b + 1) * E])
                i8s = outpool.tile([P, B * 8], U16, tag="i8s")
                nc.vector.tensor_copy(
                    i8s[:].rearrange("p (k o) -> p k o", o=1),
                    v8e[:].bitcast(U8).rearrange(
                        "p (k f) -> p k f", f=4)[:, :, 0:1])
                d.update(v8e=v8e, i8s=i8s)

            def stage_adj(sp):
                """GP: scatter column ids = idx + 256*(tile%4)."""
                d = st[sp]
                adj16 = outpool.tile([P, B * 8], I16, tag="adj16")
                nc.vector.tensor_tensor(
                    adj16[:].rearrange("p (k o) -> p k o", o=1),
                    d["v8e"][:].bitcast(U8).rearrange(
                        "p (k f) -> p k f", f=4)[:, :, 0:1],
                    offsb[:].rearrange("p (k o) -> p k o", o=1),
                    op=OP.add)
                d["adj16"] = adj16

            def stage_sc(sp):
                """GP: inverse map inv[col]=slot, then scatter bias16 by inv
                -> w9b[slot 16t+k+1] = bias[idx[t,k]]; then w8 = v8 - bias."""
                d = st[sp]
                inv = slpool.tile([P, W], I16, tag="inv")
                for h in range(2):
                    nc.gpsimd.local_scatter(
                        out_ap=inv[:, h * 1024:(h + 1) * 1024],
                        data_ap=slotb[:, h * 32:(h + 1) * 32],
                        idxs_ap=d["adj16"][:, h * 32:(h + 1) * 32],
                        channels=P, num_elems=1024, num_idxs=32)
                if SIM_SAFE_SUB:
                    nc.vector.tensor_scalar_sub(inv[:], inv[:], 1)
                w9b = slpool.tile([P, B * 16], F16, tag="w9b")
                for h in range(2):
                    nc.gpsimd.local_scatter(
                        out_ap=w9b[:, h * 64:(h + 1) * 64],
                        data_ap=biasb16[:, h * 1024:(h + 1) * 1024],
                        idxs_ap=inv[:, h * 1024:(h + 1) * 1024],
                        channels=P, num_elems=64, num_idxs=1024)
                d["w9b"] = w9b

            def stage_dens(sp):
                """DVE: w8 = v8 - bias[idx]; dens reduce + reciprocal."""
                d = st[sp]
                base = 0 if SIM_SAFE_SUB else 1
                w8 = outpool.tile([P, B * 8], F32, tag="w8")
                nc.vector.tensor_tensor(
                    w8[:].rearrange("p (t k) -> p t k", t=B),
                    d["v8e"][:].rearrange("p (t k) -> p t k", t=B),
                    d["w9b"][:].rearrange(
                        "p (t s) -> p t s", t=B)[:, :, base:base + 8],
                    op=OP.subtract)
                dens = slpool.tile([P, B], F32, tag="dens")
                nc.vector.tensor_reduce(
                    dens[:], w8[:].rearrange("p (t k) -> p t k", t=B),
                    axis=AX, op=OP.add)
                rdens = slpool.tile([P, B], F32, tag="rdens")
                nc.vector.reciprocal(rdens[:], dens[:])
                d.update(w8=w8, rdens=rdens)

            def stage_wout(sp):
                """DVE: w = w8 * 2.5 * (1/dens); store idx + w."""
                d = st.pop(sp)
                srow = sp * B * P
                wouts = outpool.tile([P, B * 8], F32, tag="wouts")
                nc.vector.scalar_tensor_tensor(
                    wouts[:].rearrange("p (t k) -> p t k", t=B),
                    d["w8"][:].rearrange("p (t k) -> p t k", t=B), 2.5,
                    d["rdens"][:].rearrange("p (t o) -> p t o", t=B)
                        .broadcast_to([P, B, 8]),
                    op0=OP.mult, op1=OP.mult)
                nc.sync.dma_start(
                    out=idx_d[srow:srow + B * P, :].rearrange(
                        "(p x) e -> p (x e)", p=P),
                    in_=d["i8s"][:])
                nc.sync.dma_start(
                    out=w_d[srow:srow + B * P, :].rearrange(
                        "(p x) e -> p (x e)", p=P),
                    in_=wouts[:])

            # pipeline lags: L/F1a +2 | gss/ranks/mask +1 | f2/adj 0 |
            #                sc -1 | dens/wout -2
            def guard(fn, sp):
                if 0 <= sp < N_SUPER:
                    fn(sp)

            guard(stage_load, 0)
            guard(stage_f1a, 0)
            guard(stage_load, 1)
            guard(stage_f1a, 1)
            guard(stage_ranks, 0)
            guard(stage_mask, 0)
            for it in range(N_SUPER + 2):
                guard(stage_load, it + 2)
                guard(stage_dens, it - 2)      # DVE + ACT heads
                guard(stage_wout, it - 2)      # GP
                guard(stage_f2, it)            # DVE: enc + Max8 + extract
                guard(stage_sc, it - 1)        # GP scatters + w8
                guard(stage_ranks, it + 1)     # DVE
                guard(stage_mask, it + 1)      # GP
                guard(stage_adj, it)           # GP
                guard(stage_f1a, it + 2)       # DVE wide block
    return nc


_COMPILED_NC = None


def _get_nc():
    global _COMPILED_NC
    if _COMPILED_NC is None:
        nc = bacc.Bacc(None, target_bir_lowering=False, debug=False)
        _build(nc)
        nc.finalize()
        _COMPILED_NC = nc
    return _COMPILED_NC


def _aux_inputs():
    offs = np.array([[256 * ((j // 8) % 4) for j in range(B * 8)]],
                    dtype=np.uint16)
    slotdat = np.array([[16 * ((j // 8) % 4) + j % 8 + 1 for j in range(B * 8)]],
                       dtype=np.int16)
    e = np.arange(W, dtype=np.uint32) % E
    erow = (np.uint32(0x3F800000) | e).view(np.float32).reshape(1, W)
    return offs, slotdat, erow


def kernel(router_logits: np.ndarray, correction_bias: np.ndarray,
           trace: bool = False):
    global LAST_EXEC_NS, LAST_RESULTS
    x = np.ascontiguousarray(np.asarray(router_logits), dtype=np.float32)
    b1 = np.ascontiguousarray(np.asarray(correction_bias),
                              dtype=np.float32).reshape(1, E)
    b = np.ascontiguousarray(np.tile(b1, (1, B)))     # [1, 2048]
    b16 = b.astype(np.float16)
    assert x.shape == (T_FULL, E), x.shape

    nc = _get_nc()
    offs, slotdat, erow = _aux_inputs()
    ident = np.eye(P, dtype=np.float32)
    ones1 = np.ones((1, P), dtype=np.float32)
    in_maps = [{"logits": x[c * T_CORE:(c + 1) * T_CORE], "bias": b,
                "bias16": b16, "erow": erow, "offs": offs,
                "slotdat": slotdat, "ident": ident, "ones1": ones1}
               for c in range(N_CORES)]
    res = run_bass_kernel_spmd(nc, in_maps, core_ids=list(range(N_CORES)),
                               trace=trace)
    LAST_EXEC_NS = res.exec_time_ns
    LAST_RESULTS = res

    idx = np.concatenate([r["idx_out"] for r in res.results], axis=0)
    w = np.concatenate([r["w_out"] for r in res.results], axis=0)
    return idx.astype(np.int32), w.astype(np.float32, copy=False)


# revision 27
# speedup vs baseline: 1.0049x; 1.0049x over previous
"""DeepseekV3 top-k router kernel for Trainium2 (Bass/Tile), 8-core SPMD.

Reference computation (per token, 256 experts):
  s    = sigmoid(logits)
  s4c  = s + correction_bias
  group scores = sum of top-2 of s4c within each of 8 groups of 32
  top-4 groups -> mask -> masked s4c
  top-8 of masked s4c -> (indices, values)
  weights = s at those indices, normalized to sum 2.5

Sharding: data-parallel on the token dim across 8 cores (16384 tokens each).
Layout: 8 tokens per SBUF partition row, 256 expert scores per token along
the free dim; supertile = [128, 2048].

Final design (411us baseline -> 353us):
  - index-encoded Max8: a fused custom DVE op (RTR_IDXENC) rewrites the
    low mantissa byte of each masked score with its expert id
    (enc = m ^ ((m ^ e) & 0xff), one pass), so Max8 alone returns
    value AND index; FindIndex8 + MATCH_VALUE_LOADs are gone.  The
    <= 2^-15 relative perturbation flips a near-tied expert on ~0.14%
    of tokens (idx rel err 1.4e-2, within the 2e-2 gate; w err 2.3e-3).
  - fused group score: custom op RTR_EQADD computes
    (s4c == max1) ? -FLT_MAX : s4c + max1 in one pass, so one segmented
    reduce yields max1+max2 directly (no separate m2 + add).
  - weights = v8 - bias16[idx]: the Max8 values ARE s4c at the selected
    experts, so scattering the constant bias row through the inverse map
    (local_scatter x2) recovers s exactly -- no second sigmoid, no
    data-dependent scatter payload.
  - s4c = sigmoid + bias via TensorE: ACT sigmoid -> SBUF, identity
    matmul + rank-1 ones x biasrow accumulate into PSUM (keeps the DVE
    out of the bias add; numerically bit-identical to the DVE add).
  - GPSIMD runs ONLY local_scatter: mixing Q7 tensor ops with scatters
    swaps the Q7 IRAM library (~2-6us per reload).  Small DVE ops stall
    behind in-flight scatters, so all tensor work stays on DVE where
    wide ops are immune.
  - 5-deep software pipeline: load+sigmoid+s4c/group block 2 ahead,
    ranks/mask 1 ahead, enc+Max8 current, scatters 1 behind,
    normalize+store 2 behind.
  """
import numpy as np

import concourse.bass as bass
import concourse.bacc as bacc
import concourse.mybir as mybir
from concourse.tile import TileContext
from concourse.bass_utils import run_bass_kernel_spmd

F32 = mybir.dt.float32
U16 = mybir.dt.uint16
F16 = mybir.dt.float16
I16 = mybir.dt.int16
U8 = mybir.dt.uint8

T_FULL = 131072
E = 256
N_CORES = 8
T_CORE = T_FULL // N_CORES      # 16384
P = 128                         # partition rows
B = 8                           # tokens per row (tiles per supertile)
N_TILES = T_CORE // P           # 128
N_SUPER = N_TILES // B          # 16
G = 8                           # expert groups per token
EG = E // G                     # experts per group
W = B * E                       # 2048 cols per supertile

SIM_SAFE_SUB = False            # CoreSim rejects duplicate scatter idxs

LAST_EXEC_NS = None
LAST_RESULTS = None

_EQNEG = None
_ENC = None
IMM_FF = float(np.frombuffer(np.uint32(255).tobytes(), np.float32)[0])


def _register(op_ctor):
    from concourse.dve_ops import (DveOp, OPS, get_dve_sub_opcode, has_src1)
    from concourse.dve_spec import lower
    from concourse.dve_uop import DveOpSpec
    import concourse.dve_ops as dve_ops_mod

    name, spec = op_ctor()
    op = DveOp(name, spec, subdim=False, uops_sha={})
    OPS.append(op)
    dve_ops_mod.CUSTOM_DVE_SPECS[op.name] = op.spec
    dve_ops_mod._SUB_OPCODE_FOR_NAME[op.name] = (
        dve_ops_mod._CUSTOM_DVE_ROW_BASE + len(OPS) - 1)
    assert dve_ops_mod._SUB_OPCODE_FOR_NAME[op.name] < 0x20
    for ver in ("v3", "v4"):
        tmp = DveOpSpec(name=op.name, opcode=get_dve_sub_opcode(op.name),
                        uops=lower(spec, ver=ver), rd1_en=has_src1(spec))
        op.uops_sha[ver] = tmp.sha(ver)
    return op


def _get_eqadd():
    """out = (in0 == in1) ? -FLT_MAX : in0 + in1 — argmax exclusion fused
    with the +max1 shift, so segmax(out) = max1 + max2 (the group score)."""
    global _EQNEG
    if _EQNEG is None:
        def ctor():
            from concourse.dve_spec import Spec, Src0, Src1, MaxNeg, select, eq

            def ref(in0, in1, s0, s1, imm2):
                b = np.broadcast_to(np.asarray(in1, np.float32),
                                    np.asarray(in0).shape)
                return np.where(
                    in0 == b, np.float32(-3.4028234663852886e38),
                    (np.asarray(in0, np.float32) + b)).astype(np.float32)

            return "RTR_EQADD", Spec(
                body=select(eq(Src0, Src1), MaxNeg, Src0 + Src1),
                reference=ref,
            )
        _EQNEG = _register(ctor)
    return _EQNEG


def _get_enc():
    """out = in0 with its low mantissa byte replaced by in1's low byte:
    enc = x ^ ((x ^ e) & 0xff) — Max8 on enc returns value AND index."""
    global _ENC
    if _ENC is None:
        def ctor():
            from concourse.dve_spec import Spec, Src0, Src1, C2, Bin, AluOp

            def ref(in0, in1, s0, s1, imm2):
                a = np.ascontiguousarray(np.asarray(in0, np.float32))
                b = np.ascontiguousarray(
                    np.broadcast_to(np.asarray(in1, np.float32), a.shape))
                ab = a.view(np.int32)
                bb = b.view(np.int32)
                m = np.array(imm2, np.float32).view(np.int32)[()]
                return (ab ^ ((ab ^ bb) & m)).view(np.float32)

            body = Bin(AluOp.BITWISE_XOR, Src0,
                       Bin(AluOp.BITWISE_AND,
                           Bin(AluOp.BITWISE_XOR, Src0, Src1), C2))
            return "RTR_IDXENC", Spec(body=body, reference=ref)
        _ENC = _register(ctor)
    return _ENC


def _build(nc: bass.Bass):
    x_d = nc.dram_tensor("logits", [T_CORE, E], F32, kind="ExternalInput")
    b_d = nc.dram_tensor("bias", [1, W], F32, kind="ExternalInput")
    id_d = nc.dram_tensor("ident", [P, P], F32, kind="ExternalInput")
    on_d = nc.dram_tensor("ones1", [1, P], F32, kind="ExternalInput")
    b16_d = nc.dram_tensor("bias16", [1, W], F16, kind="ExternalInput")
    erow_d = nc.dram_tensor("erow", [1, W], F32, kind="ExternalInput")
    offs_d = nc.dram_tensor("offs", [1, B * 8], U16, kind="ExternalInput")
    slot_d = nc.dram_tensor("slotdat", [1, B * 8], I16, kind="ExternalInput")
    idx_d = nc.dram_tensor("idx_out", [T_CORE, 8], U16, kind="ExternalOutput")
    w_d = nc.dram_tensor("w_out", [T_CORE, 8], F32, kind="ExternalOutput")

    AX = mybir.AxisListType.X
    OP = mybir.AluOpType
    ACTF = mybir.ActivationFunctionType
    eqadd = _get_eqadd()
    encop = _get_enc()

    with TileContext(nc) as tc:
        with tc.tile_pool(name="const", bufs=1) as cpool, \
             tc.tile_pool(name="io", bufs=2) as iopool, \
             tc.tile_pool(name="wideA", bufs=2) as wpool, \
             tc.tile_pool(name="s4cp", bufs=2, space="PSUM") as s4pool, \
             tc.tile_pool(name="slot", bufs=3) as slpool, \
             tc.tile_pool(name="out", bufs=4) as outpool:

            brow = cpool.tile([1, W], F32)
            nc.gpsimd.dma_start(out=brow[:], in_=b_d[:, :])
            identb = cpool.tile([P, P], F32)
            nc.gpsimd.dma_start(out=identb[:], in_=id_d[:, :])
            onesb = cpool.tile([1, P], F32)
            nc.gpsimd.dma_start(out=onesb[:], in_=on_d[:, :])
            biasb16 = cpool.tile([P, W], F16)
            nc.gpsimd.dma_start(out=biasb16[:],
                                in_=b16_d[:, :].to_broadcast((P, W)))
            erowb = cpool.tile([P, W], F32)
            nc.gpsimd.dma_start(out=erowb[:],
                                in_=erow_d[:, :].to_broadcast((P, W)))
            offsb = cpool.tile([P, B * 8], U16)
            nc.gpsimd.dma_start(out=offsb[:],
                                in_=offs_d[:, :].to_broadcast((P, B * 8)))
            slotb = cpool.tile([P, B * 8], I16)
            nc.gpsimd.dma_start(out=slotb[:],
                                in_=slot_d[:, :].to_broadcast((P, B * 8)))

            st = {}

            def stage_load(sp):
                srow = sp * B * P
                Ls = iopool.tile([P, W], F32, tag="L")
                nc.sync.dma_start(
                    out=Ls[:],
                    in_=x_d[srow:srow + B * P, :].rearrange(
                        "(p x) e -> p (x e)", p=P))
                s32 = wpool.tile([P, W], F32, tag="s32")
                nc.scalar.activation(s32[:], Ls[:], ACTF.Sigmoid)
                s4c = s4pool.tile([P, W], F32, tag="s4c")
                for n0 in range(0, W, 512):
                    nc.tensor.matmul(s4c[:, n0:n0 + 512], identb[:],
                                     s32[:, n0:n0 + 512],
                                     start=True, stop=False)
                    nc.tensor.matmul(s4c[:, n0:n0 + 512], onesb[:],
                                     brow[:, n0:n0 + 512],
                                     start=False, stop=True)
                st[sp] = dict(s32=s32, s4c=s4c)

            def stage_f1a(sp):
                """DVE block: s4c = s + bias; per-group max and 2nd max."""
                d = st[sp]
                s4c = d["s4c"]
                m1 = slpool.tile([P, B * G], F32, tag="m1")
                nc.vector.tensor_reduce(
                    m1[:].rearrange("p (b g) -> p b g", b=B),
                    s4c[:].rearrange("p (b g e) -> p b g e", b=B, g=G),
                    axis=AX, op=OP.max)
                t2 = wpool.tile([P, W], F32, tag="t2")
                nc.vector._custom_dve(
                    eqadd,
                    out=t2[:].rearrange("p (q e) -> p q e", q=B * G),
                    in0=s4c[:].rearrange("p (q e) -> p q e", q=B * G),
                    in1=m1[:].rearrange("p q -> p q", q=B * G)
                        .unsqueeze(2).broadcast_to([P, B * G, EG]))
                gss = slpool.tile([P, B * G], F32, tag="gss")
                nc.vector.tensor_reduce(
                    gss[:].rearrange("p (b g) -> p b g", b=B),
                    t2[:].rearrange("p (b g e) -> p b g e", b=B, g=G),
                    axis=AX, op=OP.max)
                d.update(gss=gss)

            def stage_ranks(sp):
                """DVE: 8x8 rank compare + rank sum."""
                d = st[sp]
                gs3 = d["gss"][:].rearrange("p (b g) -> p b g", b=B)
                gts = slpool.tile([P, B * G * G], F32, tag="gts")
                nc.vector.tensor_tensor(
                    gts[:].rearrange("p (b i j) -> p b i j", b=B, i=G),
                    gs3.unsqueeze(2).broadcast_to([P, B, G, G]),
                    gs3.unsqueeze(3).broadcast_to([P, B, G, G]),
                    op=OP.is_gt)
                ranks = slpool.tile([P, B * G], F32, tag="ranks")
                nc.vector.tensor_reduce(
                    ranks[:],
                    gts[:].rearrange("p (b i j) -> p b i j", b=B, i=G),
                    axis=AX, op=OP.add)
                d["ranks"] = ranks

            def stage_mask(sp):
                """DVE: masked = (rank < 4) * s4c in one stt pass."""
                d = st[sp]
                masked = wpool.tile([P, W], F32, tag="masked")
                nc.vector.scalar_tensor_tensor(
                    masked[:].rearrange("p (q e) -> p q e", q=B * G),
                    d["ranks"][:].rearrange("p q -> p q", q=B * G)
                        .unsqueeze(2).broadcast_to([P, B * G, EG]),
                    4.0,
                    d["s4c"][:].rearrange("p (q e) -> p q e", q=B * G),
                    op0=OP.is_lt, op1=OP.mult)
                d["masked"] = masked

            def stage_f2(sp):
                """DVE: index-encode + per-token Max8; extract expert ids."""
                d = st[sp]
                enc = wpool.tile([P, W], F32, tag="enc")
                nc.vector._custom_dve(
                    encop, out=enc[:], in0=d["masked"][:], in1=erowb[:],
                    imm2=IMM_FF)
                v8e = outpool.tile([P, B * 8], F32, tag="v8e")
                for b in range(B):
                    nc.vector.max(out=v8e[:, b * 8:(b + 1) * 8],
                                  in_=enc[:, b * E:(b + 1) * E])
                i8s = outpool.tile([P, B * 8], U16, tag="i8s")
                nc.vector.tensor_copy(
                    i8s[:].rearrange("p (k o) -> p k o", o=1),
                    v8e[:].bitcast(U8).rearrange(
                        "p (k f) -> p k f", f=4)[:, :, 0:1])
                d.update(v8e=v8e, i8s=i8s)

            def stage_adj(sp):
                """GP: scatter column ids = idx + 256*(tile%4)."""
                d = st[sp]
                adj16 = outpool.tile([P, B * 8], I16, tag="adj16")
                nc.vector.tensor_tensor(
                    adj16[:].rearrange("p (k o) -> p k o", o=1),
                    d["v8e"][:].bitcast(U8).rearrange(
                        "p (k f) -> p k f", f=4)[:, :, 0:1],
                    offsb[:].rearrange("p (k o) -> p k o", o=1),
                    op=OP.add)
                d["adj16"] = adj16

            def stage_sc(sp):
                """GP: inverse map inv[col]=slot, then scatter bias16 by inv
                -> w9b[slot 16t+k+1] = bias[idx[t,k]]; then w8 = v8 - bias."""
                d = st[sp]
                inv = slpool.tile([P, W], I16, tag="inv")
                for h in range(2):
                    nc.gpsimd.local_scatter(
                        out_ap=inv[:, h * 1024:(h + 1) * 1024],
                        data_ap=slotb[:, h * 32:(h + 1) * 32],
                        idxs_ap=d["adj16"][:, h * 32:(h + 1) * 32],
                        channels=P, num_elems=1024, num_idxs=32)
                if SIM_SAFE_SUB:
                    nc.vector.tensor_scalar_sub(inv[:], inv[:], 1)
                w9b = slpool.tile([P, B * 16], F16, tag="w9b")
                for h in range(2):
                    nc.gpsimd.local_scatter(
                        out_ap=w9b[:, h * 64:(h + 1) * 64],
                        data_ap=biasb16[:, h * 1024:(h + 1) * 1024],
                        idxs_ap=inv[:, h * 1024:(h + 1) * 1024],
                        channels=P, num_elems=64, num_idxs=1024)
                d["w9b"] = w9b

            def stage_dens(sp):
                """DVE: w8 = v8 - bias[idx]; dens reduce + reciprocal."""
                d = st[sp]
                base = 0 if SIM_SAFE_SUB else 1
                w8 = outpool.tile([P, B * 8], F32, tag="w8")
                nc.vector.tensor_tensor(
                    w8[:].rearrange("p (t k) -> p t k", t=B),
                    d["v8e"][:].rearrange("p (t k) -> p t k", t=B),
                    d["w9b"][:].rearrange(
                        "p (t s) -> p t s", t=B)[:, :, base:base + 8],
                    op=OP.subtract)
                dens = slpool.tile([P, B], F32, tag="dens")
                nc.vector.tensor_reduce(
                    dens[:], w8[:].rearrange("p (t k) -> p t k", t=B),
                    axis=AX, op=OP.add)
                rdens = slpool.tile([P, B], F32, tag="rdens")
                nc.vector.reciprocal(rdens[:], dens[:])
                d.update(w8=w8, rdens=rdens)

            def stage_wout(sp):
                """DVE: w = w8 * 2.5 * (1/dens); store idx + w."""
                d = st.pop(sp)
                srow = sp * B * P
                wouts = outpool.tile([P, B * 8], F32, tag="wouts")
                nc.vector.scalar_tensor_tensor(
                    wouts[:].rearrange("p (t k) -> p t k", t=B),
                    d["w8"][:].rearrange("p (t k) -> p t k", t=B), 2.5,
                    d["rdens"][:].rearrange("p (t o) -> p t o", t=B)
                        .broadcast_to([P, B, 8]),
                    op0=OP.mult, op1=OP.mult)
                nc.sync.dma_start(
                    out=idx_d[srow:srow + B * P, :].rearrange(
                        "(p x) e -> p (x e)", p=P),
                    in_=d["i8s"][:])
                nc.sync.dma_start(
                    out=w_d[srow:srow + B * P, :].rearrange(
                        "(p x) e -> p (x e)", p=P),
                    in_=wouts[:])

            # pipeline lags: L/F1a +2 | gss/ranks/mask +1 | f2/adj 0 |
            #                sc -1 | dens/wout -2
            def guard(fn, sp):
                if 0 <= sp < N_SUPER:
                    fn(sp)

            guard(stage_load, 0)
            guard(stage_load, 1)
            guard(stage_f1a, 0)
            guard(stage_ranks, 0)
            guard(stage_mask, 0)
            for it in range(N_SUPER + 2):
                guard(stage_f1a, it + 1)       # DVE group block (head)
                guard(stage_load, it + 2)      # DMA + ACT + TensorE s4c
                guard(stage_dens, it - 2)      # DVE + ACT heads
                guard(stage_wout, it - 2)      # GP
                guard(stage_f2, it)            # DVE: enc + Max8 + extract
                guard(stage_sc, it - 1)        # GP scatters + w8
                guard(stage_ranks, it + 1)     # DVE
                guard(stage_mask, it + 1)      # GP
                guard(stage_adj, it)           # GP
    return nc


_COMPILED_NC = None


def _get_nc():
    global _COMPILED_NC
    if _COMPILED_NC is None:
        nc = bacc.Bacc(None, target_bir_lowering=False, debug=False)
        _build(nc)
        nc.finalize()
        _COMPILED_NC = nc
    return _COMPILED_NC


def _aux_inputs():
    offs = np.array([[256 * ((j // 8) % 4) for j in range(B * 8)]],
                    dtype=np.uint16)
    slotdat = np.array([[16 * ((j // 8) % 4) + j % 8 + 1 for j in range(B * 8)]],
                       dtype=np.int16)
    e = np.arange(W, dtype=np.uint32) % E
    erow = (np.uint32(0x3F800000) | e).view(np.float32).reshape(1, W)
    return offs, slotdat, erow


def kernel(router_logits: np.ndarray, correction_bias: np.ndarray,
           trace: bool = False):
    global LAST_EXEC_NS, LAST_RESULTS
    x = np.ascontiguousarray(np.asarray(router_logits), dtype=np.float32)
    b1 = np.ascontiguousarray(np.asarray(correction_bias),
                              dtype=np.float32).reshape(1, E)
    b = np.ascontiguousarray(np.tile(b1, (1, B)))     # [1, 2048]
    b16 = b.astype(np.float16)
    assert x.shape == (T_FULL, E), x.shape

    nc = _get_nc()
    offs, slotdat, erow = _aux_inputs()
    ident = np.eye(P, dtype=np.float32)
    ones1 = np.ones((1, P), dtype=np.float32)
    in_maps = [{"logits": x[c * T_CORE:(c + 1) * T_CORE], "bias": b,
                "bias16": b16, "erow": erow, "offs": offs,
                "slotdat": slotdat, "ident": ident, "ones1": ones1}
               for c in range(N_CORES)]
    res = run_bass_kernel_spmd(nc, in_maps, core_ids=list(range(N_CORES)),
                               trace=trace)
    LAST_EXEC_NS = res.exec_time_ns
    LAST_RESULTS = res

    idx = np.concatenate([r["idx_out"] for r in res.results], axis=0)
    w = np.concatenate([r["w_out"] for r in res.results], axis=0)
    return idx.astype(np.int32), w.astype(np.float32, copy=False)


# revision 28
# speedup vs baseline: 1.2015x; 1.1956x over previous
"""DeepseekV3 top-k router kernel for Trainium2 (Bass/Tile), 8-core SPMD.

Reference computation (per token, 256 experts):
  s    = sigmoid(logits)
  s4c  = s + correction_bias
  group scores = sum of top-2 of s4c within each of 8 groups of 32
  top-4 groups -> mask -> masked s4c
  top-8 of masked s4c -> (indices, values)
  weights = s at those indices, normalized to sum 2.5

Sharding: data-parallel on the token dim across 8 cores (16384 tokens each).
Layout: 8 tokens per SBUF partition row, 256 expert scores per token along
the free dim; supertile = [128, 2048].

Final design (411us baseline -> 353us):
  - index-encoded Max8: a fused custom DVE op (RTR_IDXENC) rewrites the
    low mantissa byte of each masked score with its expert id
    (enc = m ^ ((m ^ e) & 0xff), one pass), so Max8 alone returns
    value AND index; FindIndex8 + MATCH_VALUE_LOADs are gone.  The
    <= 2^-15 relative perturbation flips a near-tied expert on ~0.14%
    of tokens (idx rel err 1.4e-2, within the 2e-2 gate; w err 2.3e-3).
  - fused group score: custom op RTR_EQADD computes
    (s4c == max1) ? -FLT_MAX : s4c + max1 in one pass, so one segmented
    reduce yields max1+max2 directly (no separate m2 + add).
  - weights = v8 - bias16[idx]: the Max8 values ARE s4c at the selected
    experts, so scattering the constant bias row through the inverse map
    (local_scatter x2) recovers s exactly -- no second sigmoid, no
    data-dependent scatter payload.
  - s4c = sigmoid + bias via TensorE: ACT sigmoid -> SBUF, identity
    matmul + rank-1 ones x biasrow accumulate into PSUM (keeps the DVE
    out of the bias add; numerically bit-identical to the DVE add).
  - GPSIMD runs ONLY local_scatter: mixing Q7 tensor ops with scatters
    swaps the Q7 IRAM library (~2-6us per reload).  Small DVE ops stall
    behind in-flight scatters, so all tensor work stays on DVE where
    wide ops are immune.
  - 5-deep software pipeline: load+sigmoid+s4c/group block 2 ahead,
    ranks/mask 1 ahead, enc+Max8 current, scatters 1 behind,
    normalize+store 2 behind.
  """
import numpy as np

import concourse.bass as bass
import concourse.bacc as bacc
import concourse.mybir as mybir
from concourse.tile import TileContext
from concourse.bass_utils import run_bass_kernel_spmd

F32 = mybir.dt.float32
U16 = mybir.dt.uint16
F16 = mybir.dt.float16
I16 = mybir.dt.int16
U8 = mybir.dt.uint8

T_FULL = 131072
E = 256
N_CORES = 8
T_CORE = T_FULL // N_CORES      # 16384
P = 128                         # partition rows
B = 8                           # tokens per row (tiles per supertile)
N_TILES = T_CORE // P           # 128
N_SUPER = N_TILES // B          # 16
G = 8                           # expert groups per token
EG = E // G                     # experts per group
W = B * E                       # 2048 cols per supertile

SIM_SAFE_SUB = False            # CoreSim rejects duplicate scatter idxs

LAST_EXEC_NS = None
LAST_RESULTS = None

_EQNEG = None
_ENC = None
IMM_FF = float(np.frombuffer(np.uint32(255).tobytes(), np.float32)[0])


def _register(op_ctor):
    from concourse.dve_ops import (DveOp, OPS, get_dve_sub_opcode, has_src1)
    from concourse.dve_spec import lower
    from concourse.dve_uop import DveOpSpec
    import concourse.dve_ops as dve_ops_mod

    name, spec = op_ctor()
    op = DveOp(name, spec, subdim=False, uops_sha={})
    OPS.append(op)
    dve_ops_mod.CUSTOM_DVE_SPECS[op.name] = op.spec
    dve_ops_mod._SUB_OPCODE_FOR_NAME[op.name] = (
        dve_ops_mod._CUSTOM_DVE_ROW_BASE + len(OPS) - 1)
    assert dve_ops_mod._SUB_OPCODE_FOR_NAME[op.name] < 0x20
    for ver in ("v3", "v4"):
        tmp = DveOpSpec(name=op.name, opcode=get_dve_sub_opcode(op.name),
                        uops=lower(spec, ver=ver), rd1_en=has_src1(spec))
        op.uops_sha[ver] = tmp.sha(ver)
    return op


def _get_eqadd():
    """out = (in0 == in1) ? -FLT_MAX : in0 + in1 — argmax exclusion fused
    with the +max1 shift, so segmax(out) = max1 + max2 (the group score)."""
    global _EQNEG
    if _EQNEG is None:
        def ctor():
            from concourse.dve_spec import Spec, Src0, Src1, MaxNeg, select, eq

            def ref(in0, in1, s0, s1, imm2):
                b = np.broadcast_to(np.asarray(in1, np.float32),
                                    np.asarray(in0).shape)
                return np.where(
                    in0 == b, np.float32(-3.4028234663852886e38),
                    (np.asarray(in0, np.float32) + b)).astype(np.float32)

            return "RTR_EQADD", Spec(
                body=select(eq(Src0, Src1), MaxNeg, Src0 + Src1),
                reference=ref,
            )
        _EQNEG = _register(ctor)
    return _EQNEG


def _get_enc():
    """out = in0 with its low mantissa byte replaced by in1's low byte:
    enc = x ^ ((x ^ e) & 0xff) — Max8 on enc returns value AND index."""
    global _ENC
    if _ENC is None:
        def ctor():
            from concourse.dve_spec import Spec, Src0, Src1, C2, Bin, AluOp

            def ref(in0, in1, s0, s1, imm2):
                a = np.ascontiguousarray(np.asarray(in0, np.float32))
                b = np.ascontiguousarray(
                    np.broadcast_to(np.asarray(in1, np.float32), a.shape))
                ab = a.view(np.int32)
                bb = b.view(np.int32)
                m = np.array(imm2, np.float32).view(np.int32)[()]
                return (ab ^ ((ab ^ bb) & m)).view(np.float32)

            body = Bin(AluOp.BITWISE_XOR, Src0,
                       Bin(AluOp.BITWISE_AND,
                           Bin(AluOp.BITWISE_XOR, Src0, Src1), C2))
            return "RTR_IDXENC", Spec(body=body, reference=ref)
        _ENC = _register(ctor)
    return _ENC


def _build(nc: bass.Bass):
    x_d = nc.dram_tensor("logits", [T_CORE, E], F32, kind="ExternalInput")
    b_d = nc.dram_tensor("bias", [1, W], F32, kind="ExternalInput")
    id_d = nc.dram_tensor("ident", [P, P], F32, kind="ExternalInput")
    on_d = nc.dram_tensor("ones1", [1, P], F32, kind="ExternalInput")
    b16_d = nc.dram_tensor("bias16", [1, W], F16, kind="ExternalInput")
    erow_d = nc.dram_tensor("erow", [1, W], F32, kind="ExternalInput")
    offs_d = nc.dram_tensor("offs", [1, B * 8], U16, kind="ExternalInput")
    slot_d = nc.dram_tensor("slotdat", [1, B * 8], I16, kind="ExternalInput")
    idx_d = nc.dram_tensor("idx_out", [T_CORE, 8], U16, kind="ExternalOutput")
    w_d = nc.dram_tensor("w_out", [T_CORE, 8], F32, kind="ExternalOutput")

    AX = mybir.AxisListType.X
    OP = mybir.AluOpType
    ACTF = mybir.ActivationFunctionType
    eqadd = _get_eqadd()
    encop = _get_enc()

    with TileContext(nc) as tc:
        with tc.tile_pool(name="const", bufs=1) as cpool, \
             tc.tile_pool(name="io", bufs=2) as iopool, \
             tc.tile_pool(name="wideA", bufs=2) as wpool, \
             tc.tile_pool(name="s4cp", bufs=2, space="PSUM") as s4pool, \
             tc.tile_pool(name="slot", bufs=3) as slpool, \
             tc.tile_pool(name="out", bufs=4) as outpool:

            brow = cpool.tile([1, W], F32)
            nc.gpsimd.dma_start(out=brow[:], in_=b_d[:, :])
            identb = cpool.tile([P, P], F32)
            nc.gpsimd.dma_start(out=identb[:], in_=id_d[:, :])
            onesb = cpool.tile([1, P], F32)
            nc.gpsimd.dma_start(out=onesb[:], in_=on_d[:, :])
            biasb16 = cpool.tile([P, W], F16)
            nc.gpsimd.dma_start(out=biasb16[:],
                                in_=b16_d[:, :].to_broadcast((P, W)))
            erowb = cpool.tile([P, W], F32)
            nc.gpsimd.dma_start(out=erowb[:],
                                in_=erow_d[:, :].to_broadcast((P, W)))
            offsb = cpool.tile([P, B * 8], U16)
            nc.gpsimd.dma_start(out=offsb[:],
                                in_=offs_d[:, :].to_broadcast((P, B * 8)))
            slotb = cpool.tile([P, B * 8], I16)
            nc.gpsimd.dma_start(out=slotb[:],
                                in_=slot_d[:, :].to_broadcast((P, B * 8)))

            st = {}

            def stage_load(sp):
                srow = sp * B * P
                Ls = iopool.tile([P, W], F32, tag="L")
                nc.sync.dma_start(
                    out=Ls[:],
                    in_=x_d[srow:srow + B * P, :].rearrange(
                        "(p x) e -> p (x e)", p=P))
                s32 = wpool.tile([P, W], F32, tag="s32")
                nc.scalar.activation(s32[:], Ls[:], ACTF.Sigmoid)
                st[sp] = dict(s32=s32)

            def stage_f1a(sp):
                """DVE block: s4c = s + bias; per-group max and 2nd max."""
                d = st[sp]
                s4c = s4pool.tile([P, W], F32, tag="s4c")
                for n0 in range(0, W, 512):
                    nc.tensor.matmul(s4c[:, n0:n0 + 512], identb[:],
                                     d["s32"][:, n0:n0 + 512],
                                     start=True, stop=False)
                    nc.tensor.matmul(s4c[:, n0:n0 + 512], onesb[:],
                                     brow[:, n0:n0 + 512],
                                     start=False, stop=True)
                m1 = slpool.tile([P, B * G], F32, tag="m1")
                nc.vector.tensor_reduce(
                    m1[:].rearrange("p (b g) -> p b g", b=B),
                    s4c[:].rearrange("p (b g e) -> p b g e", b=B, g=G),
                    axis=AX, op=OP.max)
                t2 = wpool.tile([P, W], F32, tag="t2")
                nc.vector._custom_dve(
                    eqadd,
                    out=t2[:].rearrange("p (q e) -> p q e", q=B * G),
                    in0=s4c[:].rearrange("p (q e) -> p q e", q=B * G),
                    in1=m1[:].rearrange("p q -> p q", q=B * G)
                        .unsqueeze(2).broadcast_to([P, B * G, EG]))
                gss = slpool.tile([P, B * G], F32, tag="gss")
                nc.vector.tensor_reduce(
                    gss[:].rearrange("p (b g) -> p b g", b=B),
                    t2[:].rearrange("p (b g e) -> p b g e", b=B, g=G),
                    axis=AX, op=OP.max)
                d.update(s4c=s4c, gss=gss)

            def stage_ranks(sp):
                """DVE: 8x8 rank compare + rank sum."""
                d = st[sp]
                gs3 = d["gss"][:].rearrange("p (b g) -> p b g", b=B)
                gts = slpool.tile([P, B * G * G], F32, tag="gts")
                nc.vector.tensor_tensor(
                    gts[:].rearrange("p (b i j) -> p b i j", b=B, i=G),
                    gs3.unsqueeze(2).broadcast_to([P, B, G, G]),
                    gs3.unsqueeze(3).broadcast_to([P, B, G, G]),
                    op=OP.is_gt)
                ranks = slpool.tile([P, B * G], F32, tag="ranks")
                nc.vector.tensor_reduce(
                    ranks[:],
                    gts[:].rearrange("p (b i j) -> p b i j", b=B, i=G),
                    axis=AX, op=OP.add)
                d["ranks"] = ranks

            def stage_mask(sp):
                """DVE: masked = (rank < 4) * s4c in one stt pass."""
                d = st[sp]
                masked = wpool.tile([P, W], F32, tag="masked")
                nc.vector.scalar_tensor_tensor(
                    masked[:].rearrange("p (q e) -> p q e", q=B * G),
                    d["ranks"][:].rearrange("p q -> p q", q=B * G)
                        .unsqueeze(2).broadcast_to([P, B * G, EG]),
                    4.0,
                    d["s4c"][:].rearrange("p (q e) -> p q e", q=B * G),
                    op0=OP.is_lt, op1=OP.mult)
                d["masked"] = masked

            def stage_f2(sp):
                """DVE: index-encode + per-token Max8; extract expert ids."""
                d = st[sp]
                enc = wpool.tile([P, W], F32, tag="enc")
                nc.vector._custom_dve(
                    encop, out=enc[:], in0=d["masked"][:], in1=erowb[:],
                    imm2=IMM_FF)
                v8e = outpool.tile([P, B * 8], F32, tag="v8e")
                for b in range(B):
                    nc.vector.max(out=v8e[:, b * 8:(b + 1) * 8],
                                  in_=enc[:, b * E:(b + 1) * E])
                i8s = outpool.tile([P, B * 8], U16, tag="i8s")
                nc.vector.tensor_copy(
                    i8s[:].rearrange("p (k o) -> p k o", o=1),
                    v8e[:].bitcast(U8).rearrange(
                        "p (k f) -> p k f", f=4)[:, :, 0:1])
                d.update(v8e=v8e, i8s=i8s)

            def stage_adj(sp):
                """GP: scatter column ids = idx + 256*(tile%4)."""
                d = st[sp]
                adj16 = outpool.tile([P, B * 8], I16, tag="adj16")
                nc.vector.tensor_tensor(
                    adj16[:].rearrange("p (k o) -> p k o", o=1),
                    d["v8e"][:].bitcast(U8).rearrange(
                        "p (k f) -> p k f", f=4)[:, :, 0:1],
                    offsb[:].rearrange("p (k o) -> p k o", o=1),
                    op=OP.add)
                d["adj16"] = adj16

            def stage_sc(sp):
                """GP: inverse map inv[col]=slot, then scatter bias16 by inv
                -> w9b[slot 16t+k+1] = bias[idx[t,k]]; then w8 = v8 - bias."""
                d = st[sp]
                inv = slpool.tile([P, W], I16, tag="inv")
                for h in range(2):
                    nc.gpsimd.local_scatter(
                        out_ap=inv[:, h * 1024:(h + 1) * 1024],
                        data_ap=slotb[:, h * 32:(h + 1) * 32],
                        idxs_ap=d["adj16"][:, h * 32:(h + 1) * 32],
                        channels=P, num_elems=1024, num_idxs=32)
                if SIM_SAFE_SUB:
                    nc.vector.tensor_scalar_sub(inv[:], inv[:], 1)
                w9b = slpool.tile([P, B * 16], F16, tag="w9b")
                for h in range(2):
                    nc.gpsimd.local_scatter(
                        out_ap=w9b[:, h * 64:(h + 1) * 64],
                        data_ap=biasb16[:, h * 1024:(h + 1) * 1024],
                        idxs_ap=inv[:, h * 1024:(h + 1) * 1024],
                        channels=P, num_elems=64, num_idxs=1024)
                d["w9b"] = w9b

            def stage_dens(sp):
                """DVE: w8 = v8 - bias[idx]; dens reduce + reciprocal."""
                d = st[sp]
                base = 0 if SIM_SAFE_SUB else 1
                w8 = outpool.tile([P, B * 8], F32, tag="w8")
                nc.vector.tensor_tensor(
                    w8[:].rearrange("p (t k) -> p t k", t=B),
                    d["v8e"][:].rearrange("p (t k) -> p t k", t=B),
                    d["w9b"][:].rearrange(
                        "p (t s) -> p t s", t=B)[:, :, base:base + 8],
                    op=OP.subtract)
                dens = slpool.tile([P, B], F32, tag="dens")
                nc.vector.tensor_reduce(
                    dens[:], w8[:].rearrange("p (t k) -> p t k", t=B),
                    axis=AX, op=OP.add)
                rdens = slpool.tile([P, B], F32, tag="rdens")
                nc.vector.reciprocal(rdens[:], dens[:])
                d.update(w8=w8, rdens=rdens)

            def stage_wout(sp):
                """DVE: w = w8 * 2.5 * (1/dens); store idx + w."""
                d = st.pop(sp)
                srow = sp * B * P
                wouts = outpool.tile([P, B * 8], F32, tag="wouts")
                nc.vector.scalar_tensor_tensor(
                    wouts[:].rearrange("p (t k) -> p t k", t=B),
                    d["w8"][:].rearrange("p (t k) -> p t k", t=B), 2.5,
                    d["rdens"][:].rearrange("p (t o) -> p t o", t=B)
                        .broadcast_to([P, B, 8]),
                    op0=OP.mult, op1=OP.mult)
                nc.sync.dma_start(
                    out=idx_d[srow:srow + B * P, :].rearrange(
                        "(p x) e -> p (x e)", p=P),
                    in_=d["i8s"][:])
                nc.sync.dma_start(
                    out=w_d[srow:srow + B * P, :].rearrange(
                        "(p x) e -> p (x e)", p=P),
                    in_=wouts[:])

            # pipeline lags: L/F1a +2 | gss/ranks/mask +1 | f2/adj 0 |
            #                sc -1 | dens/wout -2
            def guard(fn, sp):
                if 0 <= sp < N_SUPER:
                    fn(sp)

            guard(stage_load, 0)
            guard(stage_f1a, 0)
            guard(stage_load, 1)
            guard(stage_f1a, 1)
            guard(stage_ranks, 0)
            guard(stage_mask, 0)
            for it in range(N_SUPER + 2):
                guard(stage_load, it + 2)
                guard(stage_dens, it - 2)      # DVE + ACT heads
                guard(stage_wout, it - 2)      # GP
                guard(stage_f2, it)            # DVE: enc + Max8 + extract
                guard(stage_sc, it - 1)        # GP scatters + w8
                guard(stage_ranks, it + 1)     # DVE
                guard(stage_mask, it + 1)      # GP
                guard(stage_adj, it)           # GP
                guard(stage_f1a, it + 2)       # DVE wide block
    return nc


_COMPILED_NC = None


def _get_nc():
    global _COMPILED_NC
    if _COMPILED_NC is None:
        nc = bacc.Bacc(None, target_bir_lowering=False, debug=False)
        _build(nc)
        nc.finalize()
        _COMPILED_NC = nc
    return _COMPILED_NC


def _aux_inputs():
    offs = np.array([[256 * ((j // 8) % 4) for j in range(B * 8)]],
                    dtype=np.uint16)
    slotdat = np.array([[16 * ((j // 8) % 4) + j % 8 + 1 for j in range(B * 8)]],
                       dtype=np.int16)
    e = np.arange(W, dtype=np.uint32) % E
    erow = (np.uint32(0x3F800000) | e).view(np.float32).reshape(1, W)
    return offs, slotdat, erow


def kernel(router_logits: np.ndarray, correction_bias: np.ndarray,
           trace: bool = False):
    global LAST_EXEC_NS, LAST_RESULTS
    x = np.ascontiguousarray(np.asarray(router_logits), dtype=np.float32)
    b1 = np.ascontiguousarray(np.asarray(correction_bias),
                              dtype=np.float32).reshape(1, E)
    b = np.ascontiguousarray(np.tile(b1, (1, B)))     # [1, 2048]
    b16 = b.astype(np.float16)
    assert x.shape == (T_FULL, E), x.shape

    nc = _get_nc()
    offs, slotdat, erow = _aux_inputs()
    ident = np.eye(P, dtype=np.float32)
    ones1 = np.ones((1, P), dtype=np.float32)
    in_maps = [{"logits": x[c * T_CORE:(c + 1) * T_CORE], "bias": b,
                "bias16": b16, "erow": erow, "offs": offs,
                "slotdat": slotdat, "ident": ident, "ones1": ones1}
               for c in range(N_CORES)]
    res = run_bass_kernel_spmd(nc, in_maps, core_ids=list(range(N_CORES)),
                               trace=trace)
    LAST_EXEC_NS = res.exec_time_ns
    LAST_RESULTS = res

    idx = np.concatenate([r["idx_out"] for r in res.results], axis=0)
    w = np.concatenate([r["w_out"] for r in res.results], axis=0)
    return idx.astype(np.int32), w.astype(np.float32, copy=False)
